# revision 3
# baseline (speedup 1.0000x reference)
"""Trainium2 Bass kernel for LocalWindowAttention (B=8, C=256, H=W=64, r=32).

8-core data-parallel: one batch element per NeuronCore (attention is
independent per batch element); the small weights are replicated.

Per-core design (one batch, N=4096 tokens, 8 n-blocks x 16 m-chunk pairs):
  All PE matmuls run fp8e4m3 in DoubleRow mode (0.5 cyc/row; instruction
  cost is output-free-size driven):
    q/k proj : q' [128(4 rep groups x 32r), n512] = wq8^T @ x8, DR over
               (p,o)=c 256.  scale*log2e folded into wq (8-replica sum
               contributes the 8x).
    v proj   : vt pair tiles [m128, 2, c256] = x8-slice^T @ wv8; bias
               DROPPED (softmax rows sum to 1 so bv@attn == bv; it
               collapses to +gamma*bv[c], folded into the residual add);
               gamma folded into wv8.
    scores   : S' [m128, n512] per half-pair, lhsT = k8 m-slice, rhs = q8
               n-slice, both o-broadcast (stride-0) -> contraction = 8
               replicas of r=32; S' = 8*log2e * s_true.
    exp      : E = 2^(S'/8) per half-pair, engines alternate ACT | DVE.
               ACT: activation Exp with scale=ln2/8 -> fp8.  DVE:
               Schraudolph in e4m3 bit space: round(S' + (56 - 8*sigma))
               as int8 bitcast to fp8 (HW convert is round-to-nearest;
               the +-5% sawtooth averages out in the softmax sums;
               validated 6.4e-4 end-to-end vs the 2e-2 gate).
    colsum   : ones-DR-matmul accumulated over 16 pairs -> [1, n512].
    out      : po[h] [c128, n512] accumulating vt^T @ E over 16 pairs.

  The whole kernel is ONE global stream of pairs: scores/exp at pair g,
  out-matmuls at g-OUT_LAG, colsums at g-CS_LAG, and each block's
  normalize/residual/store tail right after its last colsum (~6 pairs
  into the next block).  Lagged consumers never make the in-order PE (or
  the DVE fifo) wait on an exp still in flight, and blocks overlap with
  no flush bubble.  PSUM: 3 half-pair score slots (3 banks) + 4 out
  accumulators (2 blocks in flight, 4 banks) + colsum (1) = 8 banks.

  Tail: recip (DVE) -> partition_broadcast (Pool, SBUF only) ->
  tmp = po*bcast (DVE) -> ot = (x + gamma*bv) + tmp (Pool) -> one packed
  [128,2,512] store per block.  Pool (gpsimd) cannot touch PSUM, so it
  only gets SBUF-only work (x fp8 copies, residual adds, broadcast).
  DMA: x loads as 4 quarter DMAs [128,2,1024] (first quarter split
  across both queues), all constants in ONE byte-blob DMA (bitcast
  views), output as 8 packed per-block stores.
"""

import numpy as np
from contextlib import ExitStack

import concourse.bass as bass
import concourse.tile as tile
from concourse import bacc, mybir, bass_utils

F32 = mybir.dt.float32
BF = mybir.dt.bfloat16
FP8 = mybir.dt.float8e4
U8 = mybir.dt.uint8
I8 = mybir.dt.int8
AF = mybir.ActivationFunctionType
ALU = mybir.AluOpType
DR = mybir.MatmulPerfMode.DoubleRow

B, C, HH, WW = 8, 256, 64, 64
N = HH * WW            # 4096 tokens
R = 32                 # low-rank q/k dim
NB = 512               # n-block (free dim per matmul)
NNB = N // NB          # 8
NPAIR = 16             # m-chunk pairs per block (2x128 tokens each)

LOG2E = 1.4426950408889634
LN2 = 0.6931471805599453
SCHRAU_SIGMA = 0.0430  # Schraudolph shift (validated on-device, RNE convert)

OUT_LAG = 4            # out-mm trails exp by this many pairs
CS_LAG = 5             # colsum trails exp

# Pairs whose BOTH exp halves go to ACT (rest are ACT,DVE):
# 19 ACT / 13 DVE halves per block, balancing ACT 0.612us/half vs DVE
# 0.658us/half + tail work.
ACT2_PAIRS = (3, 7, 12)

_cache = {}


def _build_program(nbatch=1, repeat=1):
    """repeat>1 unrolls the whole per-batch pipeline `repeat` times over
    the same DRAM buffers (virtual batches), for steady-state throughput
    measurement with a single NEFF dispatch."""
    nc = bacc.Bacc("TRN2", debug=False, num_devices=1)
    x_d = nc.dram_tensor("x", (nbatch * C, N), F32, kind="ExternalInput").ap()
    wpack_d = nc.dram_tensor("wpack", (128, 2, 512), FP8,
                             kind="ExternalInput").ap()
    onesc_d = nc.dram_tensor("onesc", (128, 2, 16), FP8,
                             kind="ExternalInput").ap()
    fpack_d = nc.dram_tensor("fpack", (128, 3), F32, kind="ExternalInput").ap()
    out_d = nc.dram_tensor("out", (nbatch * C, N), F32, kind="ExternalOutput").ap()

    with tile.TileContext(nc) as tc, ExitStack() as ctx, \
         nc.allow_low_precision(reason="fp8 matmul path"):
        consts = ctx.enter_context(tc.tile_pool(name="consts", bufs=1))
        xfp = ctx.enter_context(tc.tile_pool(name="xf", bufs=2))
        x8p = ctx.enter_context(tc.tile_pool(name="x8", bufs=2))
        qkp = ctx.enter_context(tc.tile_pool(name="qk", bufs=2))
        vtp = ctx.enter_context(tc.tile_pool(name="vt", bufs=2))
        epp = ctx.enter_context(tc.tile_pool(name="ep", bufs=10))
        misc = ctx.enter_context(tc.tile_pool(name="misc", bufs=2))

        wpack_sb = consts.tile([128, 2, 512], FP8, tag="wpack")
        nc.sync.dma_start(wpack_sb[:], wpack_d[:])
        onesc_t = consts.tile([128, 2, 16], FP8, tag="onesc")
        nc.sync.dma_start(onesc_t[:], onesc_d[:])
        fpack_sb = consts.tile([128, 3], F32, tag="fpack")
        nc.sync.dma_start(fpack_sb[:], fpack_d[:])
        wq8_sb = wpack_sb[:, :, 0:128]
        wk8_sb = wpack_sb[:, :, 128:256]
        wv8_sb = wpack_sb[:, :, 256:512]
        onesc_sb = onesc_t[:, :, 0:1]
        bq_sb = fpack_sb[:, 0:1]
        gbv_sb = fpack_sb[:, 1:3]

        pbig = ctx.enter_context(
            tc.tile_pool(name="pbig", bufs=3, space=bass.MemorySpace.PSUM))
        pop = ctx.enter_context(
            tc.tile_pool(name="pop", bufs=4, space=bass.MemorySpace.PSUM))
        psm = ctx.enter_context(
            tc.tile_pool(name="psm", bufs=1, space=bass.MemorySpace.PSUM))

        state = {}

        def load_x(b, first=False):
            """4 quarter DMAs (both c-chunks per DMA) + fp8 copies.  For the
            first batch, most copies ride the (idle) Scalar engine, which is
            2x faster at them than Pool, to shorten the fill."""
            xq = []
            x8 = []
            if first:
                dma_eng = (None, nc.scalar, nc.gpsimd, nc.sync)
                cp_eng = ("act", "pool", "dve", "act")
            else:
                dma_eng = (None, nc.sync, nc.gpsimd, nc.gpsimd)
                cp_eng = ("pool",) * 4
            for qq in range(4):
                t = xfp.tile([128, 2, 1024], F32, tag=f"xf{qq}",
                             name=f"xf_{b}_{qq}")
                bd = b % nbatch
                src = x_d[bd * C:(bd + 1) * C, qq * 1024:(qq + 1) * 1024]
                sr = src.rearrange("(o p) n -> p o n", o=2)
                x8q = x8p.tile([128, 2, 1024], FP8, tag=f"x8{qq}",
                               name=f"x8_{b}_{qq}")
                if qq == 0:
                    # split the first quarter across both queues so the
                    # first projections can start ~2x earlier
                    nc.sync.dma_start(t[:, 0, :], sr[:, 0, :])
                    nc.gpsimd.dma_start(t[:, 1, :], sr[:, 1, :])
                else:
                    dma_eng[qq].dma_start(t[:], sr)
                if cp_eng[qq] == "act":
                    nc.scalar.copy(x8q[:], t[:])
                elif cp_eng[qq] == "dve":
                    nc.vector.tensor_copy(x8q[:], t[:])
                else:
                    nc.gpsimd.tensor_copy(x8q[:], t[:])
                # fold the residual bias (gamma*bv, from dropping the v-proj
                # bias) into x in place, after the fp8 copy reads it
                for h in range(2):
                    nc.gpsimd.tensor_scalar_add(t[:, h, :], t[:, h, :],
                                                gbv_sb[:, h:h + 1])
                xq.append(t)
                x8.append(x8q)
            q8 = [qkp.tile([128, NB], FP8, tag=f"q8{s}", name=f"q8_{b}_{s}")
                  for s in range(8)]
            k8 = [qkp.tile([128, NB], FP8, tag=f"k8{s}", name=f"k8_{b}_{s}")
                  for s in range(8)]
            vt = [vtp.tile([128, 2, 256], FP8, tag=f"vt{t_}",
                           name=f"vt_{b}_{t_}") for t_ in range(NPAIR)]
            state[b] = (xq, x8, q8, k8, vt)

        def proj_step(b, step, on_act=False):
            """One projection generation through the score-slot pool.

            step 0..7   : k slice s = step      (m cols s*512..)
            step 8..15  : q slice s = step - 8  (n cols s*512..)
            step 16..31 : v pair  t = step - 16 (m chunks 2t, 2t+1)
            on_act: put the PSUM->SBUF write on the Scalar engine (copies
            only) to keep the drip off the Vector exp stream.
            """
            xq, x8, q8, k8, vt = state[b]
            if step < 16:
                is_q = step >= 8
                s = step - 8 if is_q else step
                qq, half = s // 2, s % 2
                w_sb = wq8_sb if is_q else wk8_sb
                pq = pbig.tile([128, NB], F32, tag="pbig",
                               name=f"pqk_{b}_{step}")
                nc.tensor.matmul(
                    pq[:], w_sb,
                    x8[qq][:, :, half * 512:(half + 1) * 512],
                    start=True, stop=True, perf_mode=DR)
                if is_q:
                    nc.vector.tensor_scalar_add(q8[s][:], pq[:], bq_sb)
                elif on_act:
                    nc.scalar.copy(k8[s][:], pq[:])
                else:
                    nc.vector.tensor_copy(k8[s][:], pq[:])
            else:
                t = step - 16
                qq = t // 4
                pv = pbig.tile([128, NB], F32, tag="pbig", name=f"pv_{b}_{t}")
                for u in range(2):  # m-chunk 2t + u
                    off = (t % 4) * 256 + u * 128
                    nc.tensor.matmul(
                        pv[:, u * 256:(u + 1) * 256],
                        x8[qq][:, :, off:off + 128],
                        wv8_sb, start=True, stop=True, perf_mode=DR)
                if on_act:
                    nc.scalar.copy(vt[t][:, :, :], pv[:])
                else:
                    nc.vector.tensor_copy(vt[t][:, :, :], pv[:])

        NSTEP = 32

        def stream(pairs, sched, loads):
            """Emit the global pair pipeline.

            pairs: list of (b, nb, t).  sched: {global pair index:
            [proj_step args, ...]}.  loads: {global pair index: batch}.
            """
            blk = {}          # (b, nb) -> dict(po, pc, eps)
            tails = []        # deferred tail closures, one per pair
            n = len(pairs)

            def emit_score_exp(g):
                b, nb, t = pairs[g]
                xq, x8, q8, k8, vt = state[b]
                st = blk.setdefault((b, nb), {"po": None, "pc": None,
                                              "eps": {}})
                ep = epp.tile([128, 1024], FP8, tag="ep",
                              name=f"ep_{b}_{nb}_{t}")
                st["eps"][t] = ep
                act2 = t in ACT2_PAIRS
                for o in range(2):
                    j = 2 * t + o
                    ps = pbig.tile([128, NB], F32, tag="pbig",
                                   name=f"ps_{b}_{nb}_{t}_{o}")
                    nc.tensor.matmul(
                        ps[:],
                        k8[j // 4][:, (j % 4) * 128:(j % 4 + 1) * 128]
                            .unsqueeze(1).broadcast_to((128, 2, 128)),
                        q8[nb][:].unsqueeze(1).broadcast_to((128, 2, NB)),
                        start=True, stop=True, perf_mode=DR)
                    epv = ep[:, o * NB:(o + 1) * NB]
                    if o == 0 or act2:
                        nc.scalar.activation(epv, ps[:], AF.Exp,
                                             scale=LN2 / 8.0)
                    else:
                        nc.vector.tensor_scalar_add(
                            epv.bitcast(I8), ps[:],
                            8.0 * (7.0 - SCHRAU_SIGMA))

            def emit_out(g):
                b, nb, t = pairs[g]
                st = blk[(b, nb)]
                if st["po"] is None:
                    st["po"] = [pop.tile([128, NB], F32, tag="po",
                                         name=f"po_{b}_{nb}_{h}")
                                for h in range(2)]
                _, _, _, _, vt = state[b]
                ep_r = st["eps"][t].rearrange("p (o n) -> p o n", o=2)
                for h in range(2):
                    nc.tensor.matmul(
                        st["po"][h][:],
                        vt[t][:, :, h * 128:(h + 1) * 128],
                        ep_r,
                        start=(t == 0), stop=(t == NPAIR - 1), perf_mode=DR)

            def emit_colsum(g):
                b, nb, t = pairs[g]
                st = blk[(b, nb)]
                if st["pc"] is None:
                    st["pc"] = psm.tile([128, NB], F32, tag="psm",
                                        name=f"pc_{b}_{nb}")
                ep_r = st["eps"].pop(t).rearrange("p (o n) -> p o n", o=2)
                nc.tensor.matmul(
                    st["pc"][0:1, :], onesc_sb, ep_r,
                    start=(t == 0), stop=(t == NPAIR - 1), perf_mode=DR)
                if t == NPAIR - 1:
                    start_tail(b, nb)

            def start_tail(b, nb):
                st = blk.pop((b, nb))
                po_t, pc_t = st["po"], st["pc"]
                xq = state[b][0] if b in state else st["xq"]
                recip = misc.tile([1, NB], F32, tag="recip",
                                  name=f"rc_{b}_{nb}")
                nc.vector.reciprocal(recip[:], pc_t[0:1, :])
                bcast = misc.tile([128, NB], F32, tag="bcast",
                                  name=f"bc_{b}_{nb}")
                nc.gpsimd.partition_broadcast(bcast[:], recip[:])
                ot = misc.tile([128, 2, NB], F32, tag="ot",
                               name=f"ot_{b}_{nb}")

                def half(h):
                    tmp = misc.tile([128, NB], F32, tag=f"tmp{h}",
                                    name=f"tmp_{b}_{nb}_{h}")
                    nc.vector.tensor_tensor(tmp[:], po_t[h][:], bcast[:],
                                            ALU.mult)
                    nc.gpsimd.tensor_tensor(
                        ot[:, h, :],
                        xq[nb // 2][:, h, (nb % 2) * NB:(nb % 2 + 1) * NB],
                        tmp[:], ALU.add)

                def store(h):
                    bd = b % nbatch
                    dst = out_d[bd * C + h * 128:bd * C + (h + 1) * 128,
                                nb * NB:(nb + 1) * NB]
                    nc.sync.dma_start(dst, ot[:, h, :])

                tails.extend([None, lambda: half(0),
                              lambda: (half(1), store(0)),
                              lambda: store(1)])

            # keep x tiles alive for tails that outlive state[b]
            def snapshot_xq(b):
                return state[b][0]

            for g in range(n + CS_LAG + 4):
                if g in loads:
                    load_x(loads[g])
                if g < n:
                    emit_score_exp(g)
                if g - OUT_LAG >= 0 and g - OUT_LAG < n:
                    emit_out(g - OUT_LAG)
                if g - CS_LAG >= 0 and g - CS_LAG < n:
                    emit_colsum(g - CS_LAG)
                if tails:
                    step = tails.pop(0)
                    if step is not None:
                        step()
                for args in sched.get(g, ()):
                    proj_step(*args)
            while tails:
                step = tails.pop(0)
                if step is not None:
                    step()

        # ---- fill + stream ----
        load_x(0, first=True)
        # Minimal pre-stream fill: k0, q0, q1, v0, v1.  Everything else
        # drips 2-per-pair in earliest-deadline order (k_s before pair 2s;
        # v_t before its out-mm at pair t+OUT_LAG), writes alternating
        # between Scalar and Vector.
        fill = [0, 8, 9, 16, 17]
        drip = [1, 18, 2, 19, 3, 20, 4, 21, 5, 22, 6, 23, 7, 24,
                25, 26, 27, 28, 29, 30, 31, 10]
        rest = [11, 12, 13, 14, 15]
        assert sorted(fill + drip + rest) == list(range(NSTEP))
        for s in fill:
            proj_step(0, s)
        sched = {}
        for i, s in enumerate(drip):
            sched.setdefault(i // 2, []).append((0, s, i % 3 == 0))
        for j, s in enumerate(rest):
            sched.setdefault(12 + 2 * j, []).append((0, s))
        loads = {}
        nv = nbatch * repeat
        per_batch = NNB * NPAIR
        for b in range(1, nv):
            loads[b * per_batch - 112] = b
            for i, s in enumerate(fill + drip + rest):
                sched.setdefault(b * per_batch - 100 + 3 * i, []).append(
                    (b, s, i % 3 == 0))
        pairs = [(b, nb, t) for b in range(nv)
                 for nb in range(NNB) for t in range(NPAIR)]
        stream(pairs, sched, loads)

    nc.compile()
    return nc


def _make_consts(inputs):
    import ml_dtypes
    F8NP = ml_dtypes.float8_e4m3
    wq = np.asarray(inputs["wq"], dtype=np.float32)
    bq = np.asarray(inputs["bq"], dtype=np.float32)
    wk = np.asarray(inputs["wk"], dtype=np.float32)
    wv = np.asarray(inputs["wv"], dtype=np.float32)
    bv = np.asarray(inputs["bv"], dtype=np.float32)
    gamma = float(np.asarray(inputs["gamma"]).reshape(-1)[0])
    scale = float(R) ** -0.5
    qf = scale * LOG2E
    # wq8[c', o, g*32+r] = wq[r, o*128+c'] * qf  (4 replica groups g)
    wq8 = np.tile(wq.T * qf, (1, 4)).reshape(2, 128, 128).transpose(1, 0, 2)
    wk8 = np.tile(wk.T, (1, 4)).reshape(2, 128, 128).transpose(1, 0, 2)
    # wv8[c', o, d] = wv[d, o*128+c'] * gamma
    wv8 = (wv.T * gamma).reshape(2, 128, 256).transpose(1, 0, 2)
    wpack = np.zeros((128, 2, 512), dtype=F8NP)
    wpack[:, :, 0:128] = wq8.astype(F8NP)
    wpack[:, :, 128:256] = wk8.astype(F8NP)
    wpack[:, :, 256:512] = wv8.astype(F8NP)
    onesc = np.ones((128, 2, 16), dtype=F8NP)
    fpack = np.zeros((128, 3), dtype=np.float32)
    fpack[:, 0] = np.tile(bq * qf, 4)
    fpack[:, 1:3] = (gamma * bv).reshape(2, 128).T
    return wpack, onesc, fpack


def kernel(**inputs) -> np.ndarray:
    if "prog" not in _cache:
        _cache["prog"] = _build_program(nbatch=1)
    nc = _cache["prog"]
    wpack, onesc, fpack = _make_consts(inputs)
    x = np.asarray(inputs["x"], dtype=np.float32).reshape(B * C, N)
    in_maps = [{"x": np.ascontiguousarray(x[b * C:(b + 1) * C]),
                "wpack": wpack, "onesc": onesc, "fpack": fpack}
               for b in range(B)]
    res = bass_utils.run_bass_kernel_spmd(nc, in_maps, core_ids=list(range(B)))
    out = np.stack([res.results[b]["out"] for b in range(B)])
    return out.reshape(B, C, HH, WW).astype(np.float32)


# revision 4
# speedup vs baseline: 1.0869x; 1.0869x over previous
"""Trainium2 Bass kernel for LocalWindowAttention (B=8, C=256, H=W=64, r=32).

8-core data-parallel: one batch element per NeuronCore (attention is
independent per batch element); the small weights are replicated.

Per-core design (one batch, N=4096 tokens, 8 n-blocks x 16 m-chunk pairs):
  All PE matmuls run fp8e4m3 in DoubleRow mode (0.5 cyc/row; instruction
  cost is output-free-size driven):
    q/k proj : q' [128(4 rep groups x 32r), n512] = wq8^T @ x8, DR over
               (p,o)=c 256.  scale*log2e folded into wq (8-replica sum
               contributes the 8x).
    v proj   : vt pair tiles [m128, 2, c256] = x8-slice^T @ wv8; bias
               DROPPED (softmax rows sum to 1 so bv@attn == bv; it
               collapses to +gamma*bv[c], folded into the residual add);
               gamma folded into wv8.
    scores   : S' [m128, n512] per half-pair, lhsT = k8 m-slice, rhs = q8
               n-slice, both o-broadcast (stride-0) -> contraction = 8
               replicas of r=32; S' = 8*log2e * s_true.
    exp      : E = 2^(S'/8) per half-pair, engines alternate ACT | DVE.
               ACT: activation Exp with scale=ln2/8 -> fp8.  DVE:
               Schraudolph in e4m3 bit space: round(S' + (56 - 8*sigma))
               as int8 bitcast to fp8 (HW convert is round-to-nearest;
               the +-5% sawtooth averages out in the softmax sums;
               validated 6.4e-4 end-to-end vs the 2e-2 gate).
    colsum   : ones-DR-matmul accumulated over 16 pairs -> [1, n512].
    out      : po[h] [c128, n512] accumulating vt^T @ E over 16 pairs.

  The whole kernel is ONE global stream of pairs: scores/exp at pair g,
  out-matmuls at g-OUT_LAG, colsums at g-CS_LAG, and each block's
  normalize/residual/store tail right after its last colsum (~5 pairs
  into the next block).  Lagged consumers never make the in-order PE (or
  the DVE fifo) wait on an exp still in flight, and blocks overlap with
  no flush bubble.  PSUM: 4 half-pair score slots (4 banks) + 3 out
  accumulators (4th-slot slack for the next block, 3 banks) + colsum (1)
  = 8 banks.  The tail's po-reading muls are emitted before the next
  block's second out accumulator is allocated so the 3-slot rotation
  stays stream-ordered.

  Tail: recip (DVE) -> partition_broadcast (Pool, SBUF only) ->
  tmp = po*bcast (DVE) -> ot = (x + gamma*bv) + tmp (Pool) -> one packed
  [128,2,512] store per block.  Pool (gpsimd) cannot touch PSUM, so it
  only gets SBUF-only work (x fp8 copies, residual adds, broadcast).
  DMA: x loads as 4 quarter DMAs [128,2,1024] (first quarter split
  across both queues), all constants in ONE byte-blob DMA (bitcast
  views), output as 8 packed per-block stores.
"""

import numpy as np
from contextlib import ExitStack

import concourse.bass as bass
import concourse.tile as tile
from concourse import bacc, mybir, bass_utils

F32 = mybir.dt.float32
BF = mybir.dt.bfloat16
FP8 = mybir.dt.float8e4
U8 = mybir.dt.uint8
I8 = mybir.dt.int8
AF = mybir.ActivationFunctionType
ALU = mybir.AluOpType
DR = mybir.MatmulPerfMode.DoubleRow

B, C, HH, WW = 8, 256, 64, 64
N = HH * WW            # 4096 tokens
R = 32                 # low-rank q/k dim
NB = 512               # n-block (free dim per matmul)
NNB = N // NB          # 8
NPAIR = 16             # m-chunk pairs per block (2x128 tokens each)

LOG2E = 1.4426950408889634
LN2 = 0.6931471805599453
SCHRAU_SIGMA = 0.0430  # Schraudolph shift (validated on-device, RNE convert)

OUT_LAG = 5            # out-mm trails exp by this many pairs
CS_LAG = 5             # colsum trails exp

# Pairs whose BOTH exp halves go to ACT (rest are ACT,DVE):
# 19 ACT / 13 DVE halves per block, balancing ACT 0.612us/half vs DVE
# 0.658us/half + tail work.
ACT2_PAIRS = (3, 7, 12)

_cache = {}


def _build_program(nbatch=1, repeat=1):
    """repeat>1 unrolls the whole per-batch pipeline `repeat` times over
    the same DRAM buffers (virtual batches), for steady-state throughput
    measurement with a single NEFF dispatch."""
    nc = bacc.Bacc("TRN2", debug=False, num_devices=1)
    x_d = nc.dram_tensor("x", (nbatch * C, N), F32, kind="ExternalInput").ap()
    wpack_d = nc.dram_tensor("wpack", (128, 2, 512), FP8,
                             kind="ExternalInput").ap()
    onesc_d = nc.dram_tensor("onesc", (128, 2, 16), FP8,
                             kind="ExternalInput").ap()
    fpack_d = nc.dram_tensor("fpack", (128, 3), F32, kind="ExternalInput").ap()
    out_d = nc.dram_tensor("out", (nbatch * C, N), F32, kind="ExternalOutput").ap()

    with tile.TileContext(nc) as tc, ExitStack() as ctx, \
         nc.allow_low_precision(reason="fp8 matmul path"):
        consts = ctx.enter_context(tc.tile_pool(name="consts", bufs=1))
        xfp = ctx.enter_context(tc.tile_pool(name="xf", bufs=2))
        x8p = ctx.enter_context(tc.tile_pool(name="x8", bufs=2))
        qkp = ctx.enter_context(tc.tile_pool(name="qk", bufs=2))
        vtp = ctx.enter_context(tc.tile_pool(name="vt", bufs=2))
        epp = ctx.enter_context(tc.tile_pool(name="ep", bufs=10))
        misc = ctx.enter_context(tc.tile_pool(name="misc", bufs=2))

        wpack_sb = consts.tile([128, 2, 512], FP8, tag="wpack")
        nc.sync.dma_start(wpack_sb[:], wpack_d[:])
        onesc_t = consts.tile([128, 2, 16], FP8, tag="onesc")
        nc.sync.dma_start(onesc_t[:], onesc_d[:])
        fpack_sb = consts.tile([128, 3], F32, tag="fpack")
        nc.sync.dma_start(fpack_sb[:], fpack_d[:])
        wq8_sb = wpack_sb[:, :, 0:128]
        wk8_sb = wpack_sb[:, :, 128:256]
        wv8_sb = wpack_sb[:, :, 256:512]
        onesc_sb = onesc_t[:, :, 0:1]
        bq_sb = fpack_sb[:, 0:1]
        gbv_sb = fpack_sb[:, 1:3]

        pbig = ctx.enter_context(
            tc.tile_pool(name="pbig", bufs=4, space=bass.MemorySpace.PSUM))
        pop = ctx.enter_context(
            tc.tile_pool(name="pop", bufs=3, space=bass.MemorySpace.PSUM))
        psm = ctx.enter_context(
            tc.tile_pool(name="psm", bufs=1, space=bass.MemorySpace.PSUM))

        state = {}

        def load_x(b, first=False):
            """4 quarter DMAs (both c-chunks per DMA) + fp8 copies.  For the
            first batch, most copies ride the (idle) Scalar engine, which is
            2x faster at them than Pool, to shorten the fill."""
            xq = []
            x8 = []
            if first:
                dma_eng = (None, nc.scalar, nc.gpsimd, nc.sync)
                cp_eng = ("act", "pool", "dve", "act")
            else:
                dma_eng = (None, nc.sync, nc.gpsimd, nc.gpsimd)
                cp_eng = ("pool",) * 4
            for qq in range(4):
                t = xfp.tile([128, 2, 1024], F32, tag=f"xf{qq}",
                             name=f"xf_{b}_{qq}")
                bd = b % nbatch
                src = x_d[bd * C:(bd + 1) * C, qq * 1024:(qq + 1) * 1024]
                sr = src.rearrange("(o p) n -> p o n", o=2)
                x8q = x8p.tile([128, 2, 1024], FP8, tag=f"x8{qq}",
                               name=f"x8_{b}_{qq}")
                if qq == 0 and first:
                    # split the first quarter by token halves across queues:
                    # the first projections (k0/q0/v0/v1) need only tokens
                    # 0:512, so they start as soon as the first half lands
                    nc.sync.dma_start(t[:, 0, 0:512], sr[:, 0, 0:512])
                    nc.gpsimd.dma_start(t[:, 1, 0:512], sr[:, 1, 0:512])
                    nc.scalar.copy(x8q[:, :, 0:512], t[:, :, 0:512])
                    nc.sync.dma_start(t[:, 0, 512:1024], sr[:, 0, 512:1024])
                    nc.gpsimd.dma_start(t[:, 1, 512:1024], sr[:, 1, 512:1024])
                    nc.scalar.copy(x8q[:, :, 512:1024], t[:, :, 512:1024])
                elif qq == 0:
                    nc.sync.dma_start(t[:, 0, :], sr[:, 0, :])
                    nc.gpsimd.dma_start(t[:, 1, :], sr[:, 1, :])
                    nc.gpsimd.tensor_copy(x8q[:], t[:])
                else:
                    dma_eng[qq].dma_start(t[:], sr)
                    if cp_eng[qq] == "act":
                        nc.scalar.copy(x8q[:], t[:])
                    elif cp_eng[qq] == "dve":
                        nc.vector.tensor_copy(x8q[:], t[:])
                    else:
                        nc.gpsimd.tensor_copy(x8q[:], t[:])
                # fold the residual bias (gamma*bv, from dropping the v-proj
                # bias) into x in place, after the fp8 copy reads it
                for h in range(2):
                    nc.gpsimd.tensor_scalar_add(t[:, h, :], t[:, h, :],
                                                gbv_sb[:, h:h + 1])
                xq.append(t)
                x8.append(x8q)
            q8 = [qkp.tile([128, NB], FP8, tag=f"q8{s}", name=f"q8_{b}_{s}")
                  for s in range(8)]
            k8 = [qkp.tile([128, NB], FP8, tag=f"k8{s}", name=f"k8_{b}_{s}")
                  for s in range(8)]
            vt = [vtp.tile([128, 2, 256], FP8, tag=f"vt{t_}",
                           name=f"vt_{b}_{t_}") for t_ in range(NPAIR)]
            state[b] = (xq, x8, q8, k8, vt)

        def proj_step(b, step, on_act=False):
            """One projection generation through the score-slot pool.

            step 0..7   : k slice s = step      (m cols s*512..)
            step 8..15  : q slice s = step - 8  (n cols s*512..)
            step 16..31 : v pair  t = step - 16 (m chunks 2t, 2t+1)
            on_act: put the PSUM->SBUF write on the Scalar engine (copies
            only) to keep the drip off the Vector exp stream.
            """
            xq, x8, q8, k8, vt = state[b]
            if step < 16:
                is_q = step >= 8
                s = step - 8 if is_q else step
                qq, half = s // 2, s % 2
                w_sb = wq8_sb if is_q else wk8_sb
                pq = pbig.tile([128, NB], F32, tag="pbig",
                               name=f"pqk_{b}_{step}")
                nc.tensor.matmul(
                    pq[:], w_sb,
                    x8[qq][:, :, half * 512:(half + 1) * 512],
                    start=True, stop=True, perf_mode=DR)
                if is_q:
                    nc.vector.tensor_scalar_add(q8[s][:], pq[:], bq_sb)
                elif on_act:
                    nc.scalar.copy(k8[s][:], pq[:])
                else:
                    nc.vector.tensor_copy(k8[s][:], pq[:])
            else:
                t = step - 16
                qq = t // 4
                pv = pbig.tile([128, NB], F32, tag="pbig", name=f"pv_{b}_{t}")
                for u in range(2):  # m-chunk 2t + u
                    off = (t % 4) * 256 + u * 128
                    nc.tensor.matmul(
                        pv[:, u * 256:(u + 1) * 256],
                        x8[qq][:, :, off:off + 128],
                        wv8_sb, start=True, stop=True, perf_mode=DR)
                if on_act:
                    nc.scalar.copy(vt[t][:, :, :], pv[:])
                else:
                    nc.vector.tensor_copy(vt[t][:, :, :], pv[:])

        NSTEP = 32

        def stream(pairs, sched, loads):
            """Emit the global pair pipeline.

            pairs: list of (b, nb, t).  sched: {global pair index:
            [proj_step args, ...]}.  loads: {global pair index: batch}.
            """
            blk = {}          # (b, nb) -> dict(po, pc, eps)
            tails = []        # deferred tail closures, one per pair
            n = len(pairs)

            def emit_score_exp(g):
                b, nb, t = pairs[g]
                xq, x8, q8, k8, vt = state[b]
                st = blk.setdefault((b, nb), {"po": None, "pc": None,
                                              "eps": {}})
                ep = epp.tile([128, 1024], FP8, tag="ep",
                              name=f"ep_{b}_{nb}_{t}")
                st["eps"][t] = ep
                act2 = t in ACT2_PAIRS
                for o in range(2):
                    j = 2 * t + o
                    ps = pbig.tile([128, NB], F32, tag="pbig",
                                   name=f"ps_{b}_{nb}_{t}_{o}")
                    nc.tensor.matmul(
                        ps[:],
                        k8[j // 4][:, (j % 4) * 128:(j % 4 + 1) * 128]
                            .unsqueeze(1).broadcast_to((128, 2, 128)),
                        q8[nb][:].unsqueeze(1).broadcast_to((128, 2, NB)),
                        start=True, stop=True, perf_mode=DR)
                    epv = ep[:, o * NB:(o + 1) * NB]
                    if o == 0 or act2:
                        nc.scalar.activation(epv, ps[:], AF.Exp,
                                             scale=LN2 / 8.0)
                    else:
                        nc.vector.tensor_scalar_add(
                            epv.bitcast(I8), ps[:],
                            8.0 * (7.0 - SCHRAU_SIGMA))

            def emit_out(g):
                b, nb, t = pairs[g]
                st = blk[(b, nb)]
                if st["po"] is None:
                    st["po"] = [pop.tile([128, NB], F32, tag="po",
                                         name=f"po_{b}_{nb}_{h}")
                                for h in range(2)]
                _, _, _, _, vt = state[b]
                ep_r = st["eps"][t].rearrange("p (o n) -> p o n", o=2)
                for h in range(2):
                    nc.tensor.matmul(
                        st["po"][h][:],
                        vt[t][:, :, h * 128:(h + 1) * 128],
                        ep_r,
                        start=(t == 0), stop=(t == NPAIR - 1), perf_mode=DR)

            def emit_colsum(g):
                b, nb, t = pairs[g]
                st = blk[(b, nb)]
                if st["pc"] is None:
                    st["pc"] = psm.tile([128, NB], F32, tag="psm",
                                        name=f"pc_{b}_{nb}")
                ep_r = st["eps"].pop(t).rearrange("p (o n) -> p o n", o=2)
                nc.tensor.matmul(
                    st["pc"][0:1, :], onesc_sb, ep_r,
                    start=(t == 0), stop=(t == NPAIR - 1), perf_mode=DR)
                if t == NPAIR - 1:
                    start_tail(b, nb, last=(g == n - 1 + CS_LAG))

            def start_tail(b, nb, last=False):
                st = blk.pop((b, nb))
                po_t, pc_t = st["po"], st["pc"]
                xq = state[b][0] if b in state else st["xq"]
                recip = misc.tile([1, NB], F32, tag="recip",
                                  name=f"rc_{b}_{nb}")
                nc.vector.reciprocal(recip[:], pc_t[0:1, :])
                bcast = misc.tile([128, NB], F32, tag="bcast",
                                  name=f"bc_{b}_{nb}")
                nc.gpsimd.partition_broadcast(bcast[:], recip[:])
                ot = misc.tile([128, 2, NB], F32, tag="ot",
                               name=f"ot_{b}_{nb}")

                def half(h):
                    tmp = misc.tile([128, NB], F32, tag=f"tmp{h}",
                                    name=f"tmp_{b}_{nb}_{h}")
                    nc.vector.tensor_tensor(tmp[:], po_t[h][:], bcast[:],
                                            ALU.mult)
                    # at the drain DVE is idle and faster than Pool
                    eng = nc.vector if last else nc.gpsimd
                    eng.tensor_tensor(
                        ot[:, h, :],
                        xq[nb // 2][:, h, (nb % 2) * NB:(nb % 2 + 1) * NB],
                        tmp[:], ALU.add)

                def store(h):
                    bd = b % nbatch
                    dst = out_d[bd * C + h * 128:bd * C + (h + 1) * 128,
                                nb * NB:(nb + 1) * NB]
                    if last:  # split across both queues at the drain
                        nc.sync.dma_start(dst[:, 0:256], ot[:, h, 0:256])
                        nc.gpsimd.dma_start(dst[:, 256:512], ot[:, h, 256:512])
                    else:
                        nc.sync.dma_start(dst, ot[:, h, :])

                tails.extend([lambda: (half(0), half(1)),
                              lambda: store(0), lambda: store(1)])

            # keep x tiles alive for tails that outlive state[b]
            def snapshot_xq(b):
                return state[b][0]

            for g in range(n + CS_LAG + 4):
                if g in loads:
                    load_x(loads[g])
                if g < n:
                    emit_score_exp(g)
                if g - OUT_LAG >= 0 and g - OUT_LAG < n:
                    emit_out(g - OUT_LAG)
                if g - CS_LAG >= 0 and g - CS_LAG < n:
                    emit_colsum(g - CS_LAG)
                if tails:
                    step = tails.pop(0)
                    if step is not None:
                        step()
                for args in sched.get(g, ()):
                    proj_step(*args)
            while tails:
                step = tails.pop(0)
                if step is not None:
                    step()

        # ---- fill + stream ----
        load_x(0, first=True)
        # Minimal pre-stream fill: k0, q0, q1, v0, v1.  Everything else
        # drips 2-per-pair in earliest-deadline order (k_s before pair 2s;
        # v_t before its out-mm at pair t+OUT_LAG), writes alternating
        # between Scalar and Vector.
        fill = [0, 8, 9, 16, 17]
        drip = [1, 18, 2, 19, 3, 20, 4, 21, 5, 22, 6, 23, 7, 24,
                25, 26, 27, 28, 29, 30, 31, 10]
        rest = [11, 12, 13, 14, 15]
        assert sorted(fill + drip + rest) == list(range(NSTEP))
        for s in fill:
            proj_step(0, s)
        sched = {}
        for i, s in enumerate(drip):
            sched.setdefault(i // 2, []).append((0, s, i % 3 == 0))
        for j, s in enumerate(rest):
            sched.setdefault(12 + 2 * j, []).append((0, s))
        loads = {}
        nv = nbatch * repeat
        per_batch = NNB * NPAIR
        for b in range(1, nv):
            loads[b * per_batch - 112] = b
            for i, s in enumerate(fill + drip + rest):
                sched.setdefault(b * per_batch - 100 + 3 * i, []).append(
                    (b, s, i % 3 == 0))
        pairs = [(b, nb, t) for b in range(nv)
                 for nb in range(NNB) for t in range(NPAIR)]
        stream(pairs, sched, loads)

    nc.compile()
    return nc


def _make_consts(inputs):
    import ml_dtypes
    F8NP = ml_dtypes.float8_e4m3
    wq = np.asarray(inputs["wq"], dtype=np.float32)
    bq = np.asarray(inputs["bq"], dtype=np.float32)
    wk = np.asarray(inputs["wk"], dtype=np.float32)
    wv = np.asarray(inputs["wv"], dtype=np.float32)
    bv = np.asarray(inputs["bv"], dtype=np.float32)
    gamma = float(np.asarray(inputs["gamma"]).reshape(-1)[0])
    scale = float(R) ** -0.5
    qf = scale * LOG2E
    # wq8[c', o, g*32+r] = wq[r, o*128+c'] * qf  (4 replica groups g)
    wq8 = np.tile(wq.T * qf, (1, 4)).reshape(2, 128, 128).transpose(1, 0, 2)
    wk8 = np.tile(wk.T, (1, 4)).reshape(2, 128, 128).transpose(1, 0, 2)
    # wv8[c', o, d] = wv[d, o*128+c'] * gamma
    wv8 = (wv.T * gamma).reshape(2, 128, 256).transpose(1, 0, 2)
    wpack = np.zeros((128, 2, 512), dtype=F8NP)
    wpack[:, :, 0:128] = wq8.astype(F8NP)
    wpack[:, :, 128:256] = wk8.astype(F8NP)
    wpack[:, :, 256:512] = wv8.astype(F8NP)
    onesc = np.ones((128, 2, 16), dtype=F8NP)
    fpack = np.zeros((128, 3), dtype=np.float32)
    fpack[:, 0] = np.tile(bq * qf, 4)
    fpack[:, 1:3] = (gamma * bv).reshape(2, 128).T
    return wpack, onesc, fpack


def kernel(**inputs) -> np.ndarray:
    if "prog" not in _cache:
        _cache["prog"] = _build_program(nbatch=1)
    nc = _cache["prog"]
    wpack, onesc, fpack = _make_consts(inputs)
    x = np.asarray(inputs["x"], dtype=np.float32).reshape(B * C, N)
    in_maps = [{"x": np.ascontiguousarray(x[b * C:(b + 1) * C]),
                "wpack": wpack, "onesc": onesc, "fpack": fpack}
               for b in range(B)]
    res = bass_utils.run_bass_kernel_spmd(nc, in_maps, core_ids=list(range(B)))
    out = np.stack([res.results[b]["out"] for b in range(B)])
    return out.reshape(B, C, HH, WW).astype(np.float32)


# revision 6
# speedup vs baseline: 1.1208x; 1.0312x over previous
"""Trainium2 Bass kernel for LocalWindowAttention (B=8, C=256, H=W=64, r=32).

8-core data-parallel: one batch element per NeuronCore (attention is
independent per batch element); the small weights are replicated.

Per-core design (one batch, N=4096 tokens, 8 n-blocks x 16 m-chunk pairs):
  All PE matmuls run fp8e4m3 in DoubleRow mode (0.5 cyc/row; instruction
  cost is output-free-size driven):
    q/k proj : q' [128(4 rep groups x 32r), n512] = wq8^T @ x8, DR over
               (p,o)=c 256.  scale*log2e folded into wq (8-replica sum
               contributes the 8x).
    v proj   : vt pair tiles [m128, 2, c256] = x8-slice^T @ wv8; bias
               DROPPED (softmax rows sum to 1 so bv@attn == bv; it
               collapses to +gamma*bv[c], folded into the residual add);
               gamma folded into wv8.
    scores   : S' [m128, n512] per half-pair, lhsT = k8 m-slice, rhs = q8
               n-slice, both o-broadcast (stride-0) -> contraction = 8
               replicas of r=32; S' = 8*log2e * s_true.
    exp      : E = 2^(S'/8) per half-pair, engines alternate ACT | DVE.
               ACT: activation Exp with scale=ln2/8 -> fp8.  DVE:
               Schraudolph in e4m3 bit space: round(S' + (56 - 8*sigma))
               as int8 bitcast to fp8 (HW convert is round-to-nearest;
               the +-5% sawtooth averages out in the softmax sums;
               validated 6.4e-4 end-to-end vs the 2e-2 gate).
    colsum   : ones-DR-matmul accumulated over 16 pairs -> [1, n512].
    out      : po[h] [c128, n512] accumulating vt^T @ E over 16 pairs.

  The whole kernel is ONE global stream of pairs: scores/exp at pair g,
  out-matmuls at g-OUT_LAG, colsums at g-CS_LAG, and each block's
  normalize/residual/store tail right after its last colsum (~5 pairs
  into the next block).  Lagged consumers never make the in-order PE (or
  the DVE fifo) wait on an exp still in flight, and blocks overlap with
  no flush bubble.  PSUM: 4 half-pair score slots (4 banks) + 3 out
  accumulators (4th-slot slack for the next block, 3 banks) + colsum (1)
  = 8 banks.  The tail's po-reading muls are emitted before the next
  block's second out accumulator is allocated so the 3-slot rotation
  stays stream-ordered.

  Tail: recip (DVE) -> partition_broadcast (Pool, SBUF only) ->
  tmp = po*bcast (DVE) -> ot = (x + gamma*bv) + tmp (Pool) -> one packed
  [128,2,512] store per block.  Pool (gpsimd) cannot touch PSUM, so it
  only gets SBUF-only work (x fp8 copies, residual adds, broadcast).
  DMA: x loads as 4 quarter DMAs [128,2,1024] (first quarter split
  across both queues), all constants in ONE byte-blob DMA (bitcast
  views), output as 8 packed per-block stores.
"""

import numpy as np
from contextlib import ExitStack

import concourse.bass as bass
import concourse.tile as tile
from concourse import bacc, mybir, bass_utils

F32 = mybir.dt.float32
BF = mybir.dt.bfloat16
FP8 = mybir.dt.float8e4
U8 = mybir.dt.uint8
I8 = mybir.dt.int8
AF = mybir.ActivationFunctionType
ALU = mybir.AluOpType
DR = mybir.MatmulPerfMode.DoubleRow

B, C, HH, WW = 8, 256, 64, 64
N = HH * WW            # 4096 tokens
R = 32                 # low-rank q/k dim
NB = 512               # n-block (free dim per matmul)
NNB = N // NB          # 8
NPAIR = 16             # m-chunk pairs per block (2x128 tokens each)

LOG2E = 1.4426950408889634
LN2 = 0.6931471805599453
SCHRAU_SIGMA = 0.0430  # Schraudolph shift (validated on-device, RNE convert)

OUT_LAG = 5            # out-mm trails exp by this many pairs
CS_LAG = 5             # colsum trails exp

# Pairs whose BOTH exp halves go to ACT (rest are ACT,DVE):
# 19 ACT / 13 DVE halves per block, balancing ACT 0.612us/half vs DVE
# 0.658us/half + tail work.
ACT2_PAIRS = (4, 8, 13)

_cache = {}


def _build_program(nbatch=1, repeat=1):
    """repeat>1 unrolls the whole per-batch pipeline `repeat` times over
    the same DRAM buffers (virtual batches), for steady-state throughput
    measurement with a single NEFF dispatch."""
    nc = bacc.Bacc("TRN2", debug=False, num_devices=1)
    x_d = nc.dram_tensor("x", (nbatch * C, N), F32, kind="ExternalInput").ap()
    wpack_d = nc.dram_tensor("wpack", (128, 2, 512), FP8,
                             kind="ExternalInput").ap()
    onesc_d = nc.dram_tensor("onesc", (128, 2, 16), FP8,
                             kind="ExternalInput").ap()
    fpack_d = nc.dram_tensor("fpack", (128, 3), F32, kind="ExternalInput").ap()
    out_d = nc.dram_tensor("out", (nbatch * C, N), F32, kind="ExternalOutput").ap()

    with tile.TileContext(nc) as tc, ExitStack() as ctx, \
         nc.allow_low_precision(reason="fp8 matmul path"):
        consts = ctx.enter_context(tc.tile_pool(name="consts", bufs=1))
        xfp = ctx.enter_context(tc.tile_pool(name="xf", bufs=2))
        x8p = ctx.enter_context(tc.tile_pool(name="x8", bufs=2))
        qkp = ctx.enter_context(tc.tile_pool(name="qk", bufs=2))
        vtp = ctx.enter_context(tc.tile_pool(name="vt", bufs=2))
        epp = ctx.enter_context(tc.tile_pool(name="ep", bufs=10))
        misc = ctx.enter_context(tc.tile_pool(name="misc", bufs=2))

        wpack_sb = consts.tile([128, 2, 512], FP8, tag="wpack")
        nc.gpsimd.dma_start(wpack_sb[:], wpack_d[:])
        onesc_t = consts.tile([128, 2, 16], FP8, tag="onesc")
        nc.gpsimd.dma_start(onesc_t[:], onesc_d[:])
        fpack_sb = consts.tile([128, 3], F32, tag="fpack")
        nc.gpsimd.dma_start(fpack_sb[:], fpack_d[:])
        wq8_sb = wpack_sb[:, :, 0:128]
        wk8_sb = wpack_sb[:, :, 128:256]
        wv8_sb = wpack_sb[:, :, 256:512]
        onesc_sb = onesc_t[:, :, 0:1]
        bq_sb = fpack_sb[:, 0:1]
        gbv_sb = fpack_sb[:, 1:3]

        pbig = ctx.enter_context(
            tc.tile_pool(name="pbig", bufs=4, space=bass.MemorySpace.PSUM))
        pop = ctx.enter_context(
            tc.tile_pool(name="pop", bufs=3, space=bass.MemorySpace.PSUM))
        psm = ctx.enter_context(
            tc.tile_pool(name="psm", bufs=1, space=bass.MemorySpace.PSUM))

        state = {}

        def load_x(b, first=False):
            """4 quarter DMAs (both c-chunks per DMA) + fp8 copies.  For the
            first batch, most copies ride the (idle) Scalar engine, which is
            2x faster at them than Pool, to shorten the fill."""
            xq = []
            x8 = []
            if first:
                dma_eng = (None, nc.sync, nc.sync, nc.sync)
                cp_eng = ("act", "pool", "dve", "act")
            else:
                dma_eng = (None, nc.sync, nc.gpsimd, nc.gpsimd)
                cp_eng = ("pool",) * 4
            for qq in range(4):
                t = xfp.tile([128, 2, 1024], F32, tag=f"xf{qq}",
                             name=f"xf_{b}_{qq}")
                bd = b % nbatch
                src = x_d[bd * C:(bd + 1) * C, qq * 1024:(qq + 1) * 1024]
                sr = src.rearrange("(o p) n -> p o n", o=2)
                x8q = x8p.tile([128, 2, 1024], FP8, tag=f"x8{qq}",
                               name=f"x8_{b}_{qq}")
                if qq == 0 and first:
                    # the DMA device is serial: issue the first-half token
                    # chunks first so k0/q0/v0/v1 can start at ~3.5us while
                    # the rest of x streams in behind them
                    nc.sync.dma_start(t[:, 0, 0:512], sr[:, 0, 0:512])
                    nc.sync.dma_start(t[:, 1, 0:512], sr[:, 1, 0:512])
                    nc.scalar.copy(x8q[:, :, 0:512], t[:, :, 0:512])
                    nc.sync.dma_start(t[:, 0, 512:1024], sr[:, 0, 512:1024])
                    nc.sync.dma_start(t[:, 1, 512:1024], sr[:, 1, 512:1024])
                    nc.scalar.copy(x8q[:, :, 512:1024], t[:, :, 512:1024])
                elif qq == 0:
                    nc.sync.dma_start(t[:, 0, :], sr[:, 0, :])
                    nc.gpsimd.dma_start(t[:, 1, :], sr[:, 1, :])
                    nc.gpsimd.tensor_copy(x8q[:], t[:])
                else:
                    dma_eng[qq].dma_start(t[:], sr)
                    if cp_eng[qq] == "act":
                        nc.scalar.copy(x8q[:], t[:])
                    elif cp_eng[qq] == "dve":
                        nc.vector.tensor_copy(x8q[:], t[:])
                    else:
                        nc.gpsimd.tensor_copy(x8q[:], t[:])
                # fold the residual bias (gamma*bv, from dropping the v-proj
                # bias) into x in place, after the fp8 copy reads it
                for h in range(2):
                    nc.gpsimd.tensor_scalar_add(t[:, h, :], t[:, h, :],
                                                gbv_sb[:, h:h + 1])
                xq.append(t)
                x8.append(x8q)
            q8 = [qkp.tile([128, NB], FP8, tag=f"q8{s}", name=f"q8_{b}_{s}")
                  for s in range(8)]
            k8 = [qkp.tile([128, NB], FP8, tag=f"k8{s}", name=f"k8_{b}_{s}")
                  for s in range(8)]
            vt = [vtp.tile([128, 2, 256], FP8, tag=f"vt{t_}",
                           name=f"vt_{b}_{t_}") for t_ in range(NPAIR)]
            state[b] = (xq, x8, q8, k8, vt)

        def proj_step(b, step, on_act=False):
            """One projection generation through the score-slot pool.

            step 0..7   : k slice s = step      (m cols s*512..)
            step 8..15  : q slice s = step - 8  (n cols s*512..)
            step 16..31 : v pair  t = step - 16 (m chunks 2t, 2t+1)
            on_act: put the PSUM->SBUF write on the Scalar engine (copies
            only) to keep the drip off the Vector exp stream.
            """
            xq, x8, q8, k8, vt = state[b]
            if step < 16:
                is_q = step >= 8
                s = step - 8 if is_q else step
                qq, half = s // 2, s % 2
                w_sb = wq8_sb if is_q else wk8_sb
                pq = pbig.tile([128, NB], F32, tag="pbig",
                               name=f"pqk_{b}_{step}")
                nc.tensor.matmul(
                    pq[:], w_sb,
                    x8[qq][:, :, half * 512:(half + 1) * 512],
                    start=True, stop=True, perf_mode=DR)
                if is_q:
                    nc.vector.tensor_scalar_add(q8[s][:], pq[:], bq_sb)
                elif on_act:
                    nc.scalar.copy(k8[s][:], pq[:])
                else:
                    nc.vector.tensor_copy(k8[s][:], pq[:])
            else:
                t = step - 16
                qq = t // 4
                pv = pbig.tile([128, NB], F32, tag="pbig", name=f"pv_{b}_{t}")
                for u in range(2):  # m-chunk 2t + u
                    off = (t % 4) * 256 + u * 128
                    nc.tensor.matmul(
                        pv[:, u * 256:(u + 1) * 256],
                        x8[qq][:, :, off:off + 128],
                        wv8_sb, start=True, stop=True, perf_mode=DR)
                if on_act:
                    nc.scalar.copy(vt[t][:, :, :], pv[:])
                else:
                    nc.vector.tensor_copy(vt[t][:, :, :], pv[:])

        NSTEP = 32

        def stream(pairs, sched, loads):
            """Emit the global pair pipeline.

            pairs: list of (b, nb, t).  sched: {global pair index:
            [proj_step args, ...]}.  loads: {global pair index: batch}.
            """
            blk = {}          # (b, nb) -> dict(po, pc, eps)
            tails = []        # deferred tail closures, one per pair
            n = len(pairs)

            def emit_score_exp(g):
                b, nb, t = pairs[g]
                xq, x8, q8, k8, vt = state[b]
                st = blk.setdefault((b, nb), {"po": None, "pc": None,
                                              "eps": {}})
                ep = epp.tile([128, 1024], FP8, tag="ep",
                              name=f"ep_{b}_{nb}_{t}")
                st["eps"][t] = ep
                act2 = t in ACT2_PAIRS
                for o in range(2):
                    j = 2 * t + o
                    ps = pbig.tile([128, NB], F32, tag="pbig",
                                   name=f"ps_{b}_{nb}_{t}_{o}")
                    nc.tensor.matmul(
                        ps[:],
                        k8[j // 4][:, (j % 4) * 128:(j % 4 + 1) * 128]
                            .unsqueeze(1).broadcast_to((128, 2, 128)),
                        q8[nb][:].unsqueeze(1).broadcast_to((128, 2, NB)),
                        start=True, stop=True, perf_mode=DR)
                    epv = ep[:, o * NB:(o + 1) * NB]
                    if o == 0 or act2:
                        nc.scalar.activation(epv, ps[:], AF.Exp,
                                             scale=LN2 / 8.0)
                    else:
                        nc.vector.tensor_scalar_add(
                            epv.bitcast(I8), ps[:],
                            8.0 * (7.0 - SCHRAU_SIGMA))

            def emit_out(g):
                b, nb, t = pairs[g]
                st = blk[(b, nb)]
                if st["po"] is None:
                    st["po"] = [pop.tile([128, NB], F32, tag="po",
                                         name=f"po_{b}_{nb}_{h}")
                                for h in range(2)]
                _, _, _, _, vt = state[b]
                ep_r = st["eps"][t].rearrange("p (o n) -> p o n", o=2)
                for h in range(2):
                    nc.tensor.matmul(
                        st["po"][h][:],
                        vt[t][:, :, h * 128:(h + 1) * 128],
                        ep_r,
                        start=(t == 0), stop=(t == NPAIR - 1), perf_mode=DR)

            def emit_colsum(g):
                b, nb, t = pairs[g]
                st = blk[(b, nb)]
                if st["pc"] is None:
                    st["pc"] = psm.tile([128, NB], F32, tag="psm",
                                        name=f"pc_{b}_{nb}")
                ep_r = st["eps"].pop(t).rearrange("p (o n) -> p o n", o=2)
                nc.tensor.matmul(
                    st["pc"][0:1, :], onesc_sb, ep_r,
                    start=(t == 0), stop=(t == NPAIR - 1), perf_mode=DR)
                if t == NPAIR - 1:
                    start_tail(b, nb, last=(g == n - 1 + CS_LAG))

            def start_tail(b, nb, last=False):
                st = blk.pop((b, nb))
                po_t, pc_t = st["po"], st["pc"]
                xq = state[b][0] if b in state else st["xq"]
                recip = misc.tile([1, NB], F32, tag="recip",
                                  name=f"rc_{b}_{nb}")
                nc.vector.reciprocal(recip[:], pc_t[0:1, :])
                bcast = misc.tile([128, NB], F32, tag="bcast",
                                  name=f"bc_{b}_{nb}")
                nc.gpsimd.partition_broadcast(bcast[:], recip[:])
                ot = misc.tile([128, 2, NB], F32, tag="ot",
                               name=f"ot_{b}_{nb}")

                def half(h):
                    tmp = misc.tile([128, NB], F32, tag=f"tmp{h}",
                                    name=f"tmp_{b}_{nb}_{h}")
                    nc.vector.tensor_tensor(tmp[:], po_t[h][:], bcast[:],
                                            ALU.mult)
                    # at the drain DVE is idle and faster than Pool
                    eng = nc.vector if last else nc.gpsimd
                    eng.tensor_tensor(
                        ot[:, h, :],
                        xq[nb // 2][:, h, (nb % 2) * NB:(nb % 2 + 1) * NB],
                        tmp[:], ALU.add)

                def store(h):
                    bd = b % nbatch
                    dst = out_d[bd * C + h * 128:bd * C + (h + 1) * 128,
                                nb * NB:(nb + 1) * NB]
                    if last:  # split across both queues at the drain
                        nc.sync.dma_start(dst[:, 0:256], ot[:, h, 0:256])
                        nc.gpsimd.dma_start(dst[:, 256:512], ot[:, h, 256:512])
                    else:
                        nc.sync.dma_start(dst, ot[:, h, :])

                tails.extend([lambda: (half(0), half(1)),
                              lambda: store(0), lambda: store(1)])

            # keep x tiles alive for tails that outlive state[b]
            def snapshot_xq(b):
                return state[b][0]

            for g in range(n + CS_LAG + 4):
                if g in loads:
                    load_x(loads[g])
                if g < n:
                    emit_score_exp(g)
                if g - OUT_LAG >= 0 and g - OUT_LAG < n:
                    emit_out(g - OUT_LAG)
                if g - CS_LAG >= 0 and g - CS_LAG < n:
                    emit_colsum(g - CS_LAG)
                if tails:
                    step = tails.pop(0)
                    if step is not None:
                        step()
                for args in sched.get(g, ()):
                    proj_step(*args)
            while tails:
                step = tails.pop(0)
                if step is not None:
                    step()

        # ---- fill + stream ----
        load_x(0, first=True)
        # Minimal pre-stream fill: k0, q0, q1, v0, v1.  Everything else
        # drips 2-per-pair in earliest-deadline order (k_s before pair 2s;
        # v_t before its out-mm at pair t+OUT_LAG), writes alternating
        # between Scalar and Vector.
        fill = [0, 8, 9, 16, 17]
        drip = [1, 18, 2, 19, 3, 20, 4, 21, 5, 22, 6, 23, 7, 24,
                25, 26, 27, 28, 29, 30, 31, 10]
        rest = [11, 12, 13, 14, 15]
        assert sorted(fill + drip + rest) == list(range(NSTEP))
        for s in fill:
            proj_step(0, s)
        sched = {}
        # positions must not exceed first use: k_s before score pair 2s,
        # v_t before out-mm pair t+OUT_LAG
        drip_pos = {1: 0, 18: 0, 19: 1,              # quarter-0 users
                    2: 3, 3: 4, 20: 5, 21: 5, 22: 6, 23: 6,   # x8q1
                    4: 7, 5: 8, 24: 10, 25: 10, 26: 11, 27: 11,  # x8q2
                    6: 11, 7: 12, 28: 14, 29: 14, 30: 15, 31: 15,  # x8q3
                    10: 16}
        for i, s in enumerate(drip):
            sched.setdefault(drip_pos[s], []).append((0, s, i % 3 == 0))
        for j, s in enumerate(rest):
            sched.setdefault(20 + 2 * j, []).append((0, s))
        loads = {}
        nv = nbatch * repeat
        per_batch = NNB * NPAIR
        for b in range(1, nv):
            loads[b * per_batch - 112] = b
            for i, s in enumerate(fill + drip + rest):
                sched.setdefault(b * per_batch - 100 + 3 * i, []).append(
                    (b, s, i % 3 == 0))
        pairs = [(b, nb, t) for b in range(nv)
                 for nb in range(NNB) for t in range(NPAIR)]
        stream(pairs, sched, loads)

    nc.compile()
    return nc


def _make_consts(inputs):
    import ml_dtypes
    F8NP = ml_dtypes.float8_e4m3
    wq = np.asarray(inputs["wq"], dtype=np.float32)
    bq = np.asarray(inputs["bq"], dtype=np.float32)
    wk = np.asarray(inputs["wk"], dtype=np.float32)
    wv = np.asarray(inputs["wv"], dtype=np.float32)
    bv = np.asarray(inputs["bv"], dtype=np.float32)
    gamma = float(np.asarray(inputs["gamma"]).reshape(-1)[0])
    scale = float(R) ** -0.5
    qf = scale * LOG2E
    # wq8[c', o, g*32+r] = wq[r, o*128+c'] * qf  (4 replica groups g)
    wq8 = np.tile(wq.T * qf, (1, 4)).reshape(2, 128, 128).transpose(1, 0, 2)
    wk8 = np.tile(wk.T, (1, 4)).reshape(2, 128, 128).transpose(1, 0, 2)
    # wv8[c', o, d] = wv[d, o*128+c'] * gamma
    wv8 = (wv.T * gamma).reshape(2, 128, 256).transpose(1, 0, 2)
    wpack = np.zeros((128, 2, 512), dtype=F8NP)
    wpack[:, :, 0:128] = wq8.astype(F8NP)
    wpack[:, :, 128:256] = wk8.astype(F8NP)
    wpack[:, :, 256:512] = wv8.astype(F8NP)
    onesc = np.ones((128, 2, 16), dtype=F8NP)
    fpack = np.zeros((128, 3), dtype=np.float32)
    fpack[:, 0] = np.tile(bq * qf, 4)
    fpack[:, 1:3] = (gamma * bv).reshape(2, 128).T
    return wpack, onesc, fpack


def kernel(**inputs) -> np.ndarray:
    if "prog" not in _cache:
        _cache["prog"] = _build_program(nbatch=1)
    nc = _cache["prog"]
    wpack, onesc, fpack = _make_consts(inputs)
    x = np.asarray(inputs["x"], dtype=np.float32).reshape(B * C, N)
    in_maps = [{"x": np.ascontiguousarray(x[b * C:(b + 1) * C]),
                "wpack": wpack, "onesc": onesc, "fpack": fpack}
               for b in range(B)]
    res = bass_utils.run_bass_kernel_spmd(nc, in_maps, core_ids=list(range(B)))
    out = np.stack([res.results[b]["out"] for b in range(B)])
    return out.reshape(B, C, HH, WW).astype(np.float32)
